# revision 1
# baseline (speedup 1.0000x reference)
"""Trainium2 Bass kernel for DFine multi-head attention.

Problem: B=2, S=2048, D=1024, H=16 heads, HD=64.
Sharding over 8 cores: core c handles batch b=c//4 and head-group g=c%4
(4 heads). Each core computes its heads' attention and a partial
out-projection [2048, 1024]; the host sums the 4 partials per batch and
adds the output bias.

All matmuls run in float32r (TF32-like, full PE rate for moving dim
>= 256, ~1.5e-4 relative error).
"""

import sys
import numpy as np

if "/opt/trn_rl_repo" not in sys.path:
    sys.path.insert(0, "/opt/trn_rl_repo")

B, S, D, H, HD = 2, 2048, 1024, 16, 64
G = 4          # heads per core
E = G * HD     # 256 per-core head width
T = S          # tokens
KC = 8         # contraction chunks of 128 over D
TB = 512       # t-block (moving free dim)
NT = T // TB   # 4
NS = T // 128  # 16 s-chunks
SCALE = HD ** -0.5

_PROGRAM = None


def _build_program(reps=1):
    import concourse.bacc as bacc
    import concourse.tile as tile
    from concourse import mybir

    f32 = mybir.dt.float32

    nc = bacc.Bacc("TRN2", target_bir_lowering=False, debug=False)

    xT_d = nc.declare_dram_parameter("xT", [D, T], f32, isOutput=False)
    pT_d = nc.declare_dram_parameter("pT", [D, T], f32, isOutput=False)
    wq_d = nc.declare_dram_parameter("wq", [D, E], f32, isOutput=False)
    wk_d = nc.declare_dram_parameter("wk", [D, E], f32, isOutput=False)
    wv_d = nc.declare_dram_parameter("wv", [D, E], f32, isOutput=False)
    wo_d = nc.declare_dram_parameter("wo", [E, D], f32, isOutput=False)
    bq_d = nc.declare_dram_parameter("bq", [2, 128, 1], f32, isOutput=False)
    bk_d = nc.declare_dram_parameter("bk", [2, 128, 1], f32, isOutput=False)
    bv_d = nc.declare_dram_parameter("bvr", [128, E], f32, isOutput=False)
    out_d = nc.declare_dram_parameter("out", [T, D], f32, isOutput=True)

    with tile.TileContext(nc) as tc:
        for rep in range(reps):
            _build_body(nc, tc, mybir, rep,
                        (xT_d, pT_d, wq_d, wk_d, wv_d, wo_d, bq_d, bk_d,
                         bv_d, out_d))

    nc.compile()
    return nc


def _build_body(nc, tc, mybir, rep, drams):
    from contextlib import ExitStack

    fr = mybir.dt.float32r
    f32 = mybir.dt.float32
    Exp = mybir.ActivationFunctionType.Exp
    (xT_d, pT_d, wq_d, wk_d, wv_d, wo_d, bq_d, bk_d, bv_d, out_d) = drams
    R = f"r{rep}_"

    octx = ExitStack()
    wpool = octx.enter_context(tc.tile_pool(name=f"{R}wpool", bufs=1))
    qkpool = octx.enter_context(tc.tile_pool(name=f"{R}qkpool", bufs=1))
    vpool = octx.enter_context(tc.tile_pool(name=f"{R}vpool", bufs=1))

    # ---- persistent tiles ----
    wq_t = wpool.tile([128, KC, E], fr, name=f"{R}wq_t")
    wk_t = wpool.tile([128, KC, E], fr, name=f"{R}wk_t")
    wv_t = wpool.tile([128, KC, E], fr, name=f"{R}wv_t")
    bq_t = wpool.tile([128, 2, 1], f32, name=f"{R}bq_t")
    bk_t = wpool.tile([128, 2, 1], f32, name=f"{R}bk_t")
    bv_t = wpool.tile([128, E], f32, name=f"{R}bv_t")
    ones_f = wpool.tile([1, 64], f32, name=f"{R}ones_f")
    ones_r = wpool.tile([1, 64], fr, name=f"{R}ones_r")
    oneblk = wpool.tile([128, NS, G, 1], f32, name=f"{R}oneblk")

    qT = [qkpool.tile([128, T], fr, name=f"{R}qT{p}") for p in range(2)]
    kT = [qkpool.tile([128, T], fr, name=f"{R}kT{p}") for p in range(2)]
    v_aug = vpool.tile([128, NS, G, HD + 1], fr, name=f"{R}v_aug")

    # ---- weight / bias DMAs (first: v-proj needs wv immediately) ----
    nc.gpsimd.dma_start(
        wv_t[:], wv_d[:].bitcast(fr).rearrange("(c p) e -> p c e", p=128))
    nc.gpsimd.dma_start(
        wq_t[:], wq_d[:].bitcast(fr).rearrange("(c p) e -> p c e", p=128))
    nc.gpsimd.dma_start(
        wk_t[:], wk_d[:].bitcast(fr).rearrange("(c p) e -> p c e", p=128))
    nc.gpsimd.dma_start(bq_t[:], bq_d[:].rearrange("c p o -> p c o"))
    nc.gpsimd.dma_start(bk_t[:], bk_d[:].rearrange("c p o -> p c o"))
    nc.gpsimd.dma_start(bv_t[:], bv_d[:])
    nc.vector.memset(ones_f[:], 1.0)
    nc.vector.tensor_copy(ones_r[:], ones_f[:])
    nc.vector.memset(oneblk[:], 1.0)
    nc.vector.tensor_copy(v_aug[:, :, :, HD:HD + 1], oneblk[:])

    # ---- phase A/B: projections (DMA-overlapped, k-outer) ----
    ictx = ExitStack()
    ppool = ictx.enter_context(tc.tile_pool(name=f"{R}ppool", bufs=1))
    hT_t = ppool.tile([128, KC, T], fr, name=f"{R}hT_t")
    qkps = ictx.enter_context(tc.tile_pool(name=f"{R}qkps", bufs=1,
                                           space="PSUM"))

    actx = ExitStack()
    xpool = actx.enter_context(tc.tile_pool(name=f"{R}xpool", bufs=1))
    pps = actx.enter_context(tc.tile_pool(name=f"{R}pps", bufs=1,
                                          space="PSUM"))
    xT_t = xpool.tile([128, KC, T], fr, name=f"{R}xT_t")
    for k in range(KC):
        nc.sync.dma_start(
            xT_t[:, k, :], xT_d[:].bitcast(fr)[k * 128:(k + 1) * 128, :])
        nc.sync.dma_start(
            hT_t[:, k, :], pT_d[:].bitcast(fr)[k * 128:(k + 1) * 128, :])

    # hT = xT + pT in place on the pT tiles (gated only by the two DMAs);
    # two half-adds per chunk so the first q/k k-step unblocks sooner
    for k in range(KC):
        for hf in range(2):
            sl = slice(hf * (T // 2), (hf + 1) * (T // 2))
            nc.vector.tensor_tensor(
                hT_t[:, k, sl], hT_t[:, k, sl], xT_t[:, k, sl],
                op=mybir.AluOpType.add)

    # q/k projections, k-outer. Pair-0 (the phase-C critical path) gets
    # all 8 concurrent psum groups: q in pps tags 0-3, k in pps 4-5 +
    # the two long-lived qkps tags. Pair-1 is emitted mid-phase-C.
    def qk_wave(w_t, b_t, dsts, nm, p, tbs, slots=None):
        pss = {}
        for i, tb in enumerate(tbs):
            if slots is None:
                pss[tb] = qkps.tile([128, TB], f32,
                                    name=f"{R}{nm}ps{p}{tb}",
                                    tag=f"qk{tb % 2}")
            else:
                pool, tag = slots[i]
                pss[tb] = pool.tile([128, TB], f32,
                                    name=f"{R}{nm}ps{p}{tb}", tag=tag)
        for k in range(KC):
            for tb in tbs:
                nc.tensor.matmul(
                    pss[tb][:],
                    w_t[:, k, p * 128:(p + 1) * 128],
                    hT_t[:, k, tb * TB:(tb + 1) * TB],
                    start=(k == 0), stop=(k == KC - 1))
        for tb in tbs:
            nc.scalar.activation(
                dsts[p][:, tb * TB:(tb + 1) * TB], pss[tb][:],
                mybir.ActivationFunctionType.Identity, bias=b_t[:, p, :])

    # v projection emission happens in phase C (after attention_pair(0,0))
    # so its matmuls fill PE under the ACT-bound stretch; defined here for
    # access to xT/wv tiles. 4-chunk psum windows on the 2 qkps banks,
    # si-outer so slice si completes just ahead of attnV's demand.
    def v_proj():
        for si in range(NS):
            for w in range(2):
                ps = qkps.tile([128, E], f32, name=f"{R}vp{w}_{si}",
                               tag=f"qk{w}")
                for kk in range(4):
                    k = w * 4 + kk
                    nc.tensor.matmul(
                        ps[:], xT_t[:, k, si * 128:(si + 1) * 128],
                        wv_t[:, k, :], start=(kk == 0), stop=(kk == 3))
                dst = v_aug[:, si, :, 0:HD]
                psg = ps[:].rearrange("p (g e) -> p g e", g=G)
                if w == 0:
                    nc.vector.tensor_tensor(
                        dst, psg, bv_t[:].rearrange("p (g e) -> p g e", g=G),
                        op=mybir.AluOpType.add)
                else:
                    nc.vector.tensor_tensor(dst, dst, psg,
                                            op=mybir.AluOpType.add)

    v_proj()
    qk_wave(wq_t, bq_t, qT, "q", 0, (0, 1, 2, 3),
            slots=[(pps, f"t{i}") for i in range(4)])
    qk_wave(wk_t, bk_t, kT, "k", 0, (0, 1, 2, 3),
            slots=[(pps, "t4"), (pps, "t5"), (qkps, "qk0"), (qkps, "qk1")])
    actx.close()  # frees xT + the 6-bank pair-0 psum pool

    # ---- phase C/D: attention + out-projection ----
    cctx = ExitStack()
    a2pool = cctx.enter_context(tc.tile_pool(name=f"{R}a2pool", bufs=1))
    epool = cctx.enter_context(tc.tile_pool(name=f"{R}epool", bufs=7))
    npool = cctx.enter_context(tc.tile_pool(name=f"{R}npool", bufs=2))
    opool = cctx.enter_context(tc.tile_pool(name=f"{R}opool", bufs=2))
    scps = cctx.enter_context(tc.tile_pool(name=f"{R}scps", bufs=2,
                                           space="PSUM"))
    atps = cctx.enter_context(tc.tile_pool(name=f"{R}atps", bufs=1,
                                           space="PSUM"))

    at2 = [a2pool.tile([128, T], fr, name=f"{R}at2_{p}") for p in range(2)]
    wo_t = a2pool.tile([128, 2, D], fr, name=f"{R}wo_t")
    nc.gpsimd.dma_start(
        wo_t[:], wo_d[:].bitcast(fr).rearrange("(c p) d -> p c d", p=128))

    def attention_pair(tb, p):
        t0 = tb * TB
        atp = [atps.tile([HD + 1, TB], f32, name=f"{R}at_{tb}_{p}_{h}",
                         tag=f"at{h}") for h in range(2)]
        for si in range(NS):
            scp = scps.tile([128, 2, TB], f32,
                            name=f"{R}sc_{tb}_{p}_{si}", tag="sc")
            for h in range(2):
                nc.tensor.matmul(
                    scp[:, h, :],
                    kT[p][h * 64:(h + 1) * 64, si * 128:(si + 1) * 128],
                    qT[p][h * 64:(h + 1) * 64, t0:t0 + TB],
                    start=True, stop=True)
            ex = epool.tile([128, 2, TB], fr,
                            name=f"{R}ex_{tb}_{p}_{si}", tag="exp")
            nc.scalar.activation(ex[:], scp[:], Exp)
            for h in range(2):
                nc.tensor.matmul(
                    atp[h][:],
                    v_aug[:, si, p * 2 + h, :],
                    ex[:, h, :],
                    start=(si == 0), stop=(si == NS - 1),
                    skip_group_check=True)
        # normalize heads of this pair; bc reuses the freed at-slot
        for h in range(2):
            rec = npool.tile([1, TB], fr, name=f"{R}rc_{tb}_{p}_{h}",
                             tag="rec")
            with nc.allow_low_precision(reason="f32r recip"):
                nc.vector.reciprocal(rec[:], atp[h][HD:HD + 1, :])
            a2s = at2[p][h * 64:(h + 1) * 64, t0:t0 + TB]
            nc.vector.tensor_copy(a2s, atp[h][0:HD, :])
            bc = atps.tile([64, TB], f32, name=f"{R}bc_{tb}_{p}_{h}",
                           tag=f"at{h}")
            nc.tensor.matmul(bc[:], ones_r[:], rec[:], start=True, stop=True)
            nc.vector.tensor_tensor(a2s, a2s, bc[:],
                                    op=mybir.AluOpType.mult)

    def out_proj(tb):
        t0 = tb * TB
        for ts in range(TB // 128):
            tsl = t0 + ts * 128
            osb = opool.tile([128, D], f32, name=f"{R}osb_{tb}_{ts}",
                             tag="osb")
            for dc in range(2):
                ps = qkps.tile([128, 512], f32, name=f"{R}op_{tb}_{ts}_{dc}",
                               tag=f"qk{dc}")
                for p in range(2):
                    nc.tensor.matmul(
                        ps[:], at2[p][:, tsl:tsl + 128],
                        wo_t[:, p, dc * 512:(dc + 1) * 512],
                        start=(p == 0), stop=(p == 1))
                nc.vector.tensor_copy(osb[:, dc * 512:(dc + 1) * 512], ps[:])
            nc.sync.dma_start(out_d[tsl:tsl + 128, :], osb[:])

    qk_wave(wq_t, bq_t, qT, "q", 1, (0, 1))
    qk_wave(wq_t, bq_t, qT, "q", 1, (2, 3))
    attention_pair(0, 0)
    # k pair-1: low priority, fills PE idle under ACT during A(0,0)
    qk_wave(wk_t, bk_t, kT, "k", 1, (0, 1))
    qk_wave(wk_t, bk_t, kT, "k", 1, (2, 3))
    attention_pair(0, 1)
    for tb in range(1, NT):
        attention_pair(tb, 0)
        out_proj(tb - 1)
        attention_pair(tb, 1)
    out_proj(NT - 1)

    cctx.close()
    ictx.close()  # frees hT + qk psum
    octx.close()



def _get_program(reps=1):
    global _PROGRAM
    if _PROGRAM is None:
        _PROGRAM = {}
    if reps not in _PROGRAM:
        _PROGRAM[reps] = _build_program(reps)
    return _PROGRAM[reps]


def _shard_inputs(inputs):
    """Build the 8 per-core input maps from the full-problem inputs."""
    hs = np.asarray(inputs["hidden_states"], np.float32)
    pe = np.asarray(inputs["position_embeddings"], np.float32)
    Wq = np.asarray(inputs["Wq"], np.float32).reshape(D, H * HD)
    Wk = np.asarray(inputs["Wk"], np.float32).reshape(D, H * HD)
    Wv = np.asarray(inputs["Wv"], np.float32).reshape(D, H * HD)
    Wo = np.asarray(inputs["Wo"], np.float32)
    bq = np.asarray(inputs["bq"], np.float32).reshape(H * HD)
    bk = np.asarray(inputs["bk"], np.float32).reshape(H * HD)
    bv = np.asarray(inputs["bv"], np.float32).reshape(H * HD)

    xT = [np.ascontiguousarray(hs[b].T) for b in range(B)]
    pT = [np.ascontiguousarray(pe[b].T) for b in range(B)]

    in_maps = []
    for c in range(8):
        b, g = divmod(c, G)
        sel = slice(g * E, (g + 1) * E)
        in_maps.append({
            "xT": xT[b],
            "pT": pT[b],
            "wq": np.ascontiguousarray(Wq[:, sel]) * np.float32(SCALE),
            "wk": np.ascontiguousarray(Wk[:, sel]),
            "wv": np.ascontiguousarray(Wv[:, sel]),
            "wo": np.ascontiguousarray(Wo[sel, :]),
            "bq": (bq[sel] * np.float32(SCALE)).reshape(2, 128, 1).copy(),
            "bk": bk[sel].reshape(2, 128, 1).copy(),
            "bvr": np.tile(bv[sel][None, :], (128, 1)),
        })
    return in_maps


def _gather_outputs(results, inputs):
    bo = np.asarray(inputs["bo"], np.float32)
    out = np.empty((B, S, D), np.float32)
    for b in range(B):
        acc = results[4 * b]["out"].astype(np.float32).copy()
        for g in range(1, G):
            acc += results[4 * b + g]["out"]
        out[b] = acc + bo[None, :]
    return out


def kernel(**inputs):
    from concourse.bass_utils import run_bass_kernel_spmd

    nc = _get_program()
    in_maps = _shard_inputs(inputs)
    res = run_bass_kernel_spmd(nc, in_maps, list(range(8)))
    return _gather_outputs(res.results, inputs)



# revision 20
# speedup vs baseline: 1.2572x; 1.2572x over previous
"""Trainium2 Bass kernel for DFine multi-head attention.

Problem: B=2, S=2048, D=1024, H=16 heads, HD=64.
Sharding over 8 cores: core c handles batch b=c//4 and head-group g=c%4
(4 heads). Each core computes its heads' attention and a partial
out-projection [2048, 1024]; the host sums the 4 partials per batch and
adds the output bias.

v3: software-pipelined attention (scores run one s-chunk ahead of
attn*V), bf16 activations/weights from the host (halves input DMA,
h=x+pos precomputed on host), t-sliced input DMAs so projections start
at ~6us, projection / v / out-proj work interleaved into the attention
loop as PE filler via a keyed work queue with a shared per-step row
budget, attn*V accumulated in two 8-chunk PSUM blocks flushed to an
SBUF accumulator by the DVE so the next unit never waits on the
normalization chain, out-proj PSUM->SBUF staging on the idle Pool
engine, normalization fused into one scalar_tensor_tensor.
"""

import sys
import numpy as np

if "/opt/trn_rl_repo" not in sys.path:
    sys.path.insert(0, "/opt/trn_rl_repo")

B, S, D, H, HD = 2, 2048, 1024, 16, 64
G = 4          # heads per core
E = G * HD     # 256 per-core head width
T = S          # tokens
KC = 8         # contraction chunks of 128 over D
TB = 512       # t-block (moving free dim)
NT = T // TB   # 4 t-blocks
NS = T // 128  # 16 s-chunks
SCALE = HD ** -0.5

_PROGRAM = None


def _build_program(reps=1):
    import concourse.bacc as bacc
    import concourse.tile as tile
    from concourse import mybir

    f32 = mybir.dt.float32
    bf16 = mybir.dt.bfloat16

    nc = bacc.Bacc("TRN2", target_bir_lowering=False, debug=False)

    xT_d = nc.declare_dram_parameter("xT", [D, T], bf16, isOutput=False)
    hT_d = nc.declare_dram_parameter("hT", [D, T], bf16, isOutput=False)
    wq_d = nc.declare_dram_parameter("wq", [D, E], bf16, isOutput=False)
    wk_d = nc.declare_dram_parameter("wk", [D, E], bf16, isOutput=False)
    wv_d = nc.declare_dram_parameter("wv", [D, E], bf16, isOutput=False)
    wo_d = nc.declare_dram_parameter("wo", [E, D], f32, isOutput=False)
    bq_d = nc.declare_dram_parameter("bq", [2, 128, 1], f32, isOutput=False)
    bk_d = nc.declare_dram_parameter("bk", [2, 128, 1], f32, isOutput=False)
    bv_d = nc.declare_dram_parameter("bvr", [128, E], f32, isOutput=False)
    out_d = nc.declare_dram_parameter("out", [T, D], f32, isOutput=True)

    with tile.TileContext(nc) as tc:
        for rep in range(reps):
            _build_body(nc, tc, mybir, rep,
                        (xT_d, hT_d, wq_d, wk_d, wv_d, wo_d, bq_d, bk_d,
                         bv_d, out_d))

    nc.compile()
    return nc


def _build_body(nc, tc, mybir, rep, drams):
    from contextlib import ExitStack

    fr = mybir.dt.float32r
    f32 = mybir.dt.float32
    bf16 = mybir.dt.bfloat16
    Exp = mybir.ActivationFunctionType.Exp
    Ident = mybir.ActivationFunctionType.Identity
    mult = mybir.AluOpType.mult
    addop = mybir.AluOpType.add
    (xT_d, hT_d, wq_d, wk_d, wv_d, wo_d, bq_d, bk_d, bv_d, out_d) = drams
    R = f"r{rep}_"

    octx = ExitStack()
    wpool = octx.enter_context(tc.tile_pool(name=f"{R}wpool", bufs=1))
    dpool = octx.enter_context(tc.tile_pool(name=f"{R}dpool", bufs=1))
    apool = octx.enter_context(tc.tile_pool(name=f"{R}apool", bufs=1))
    expool = octx.enter_context(tc.tile_pool(name=f"{R}expool", bufs=3))
    ospool = octx.enter_context(tc.tile_pool(name=f"{R}ospool", bufs=2))
    rpool = octx.enter_context(tc.tile_pool(name=f"{R}rpool", bufs=2))
    accpool = octx.enter_context(tc.tile_pool(name=f"{R}accpool", bufs=2))
    qkps = octx.enter_context(tc.tile_pool(name=f"{R}qkps", bufs=1,
                                           space="PSUM"))

    # ---- persistent tiles ----
    wq_t = wpool.tile([128, KC, E], bf16, name=f"{R}wq_t")
    wk_t = wpool.tile([128, KC, E], bf16, name=f"{R}wk_t")
    wv_t = wpool.tile([128, KC, E], bf16, name=f"{R}wv_t")
    wo_t = wpool.tile([128, 2, D], fr, name=f"{R}wo_t")
    bq_t = wpool.tile([128, 2, 1], f32, name=f"{R}bq_t")
    bk_t = wpool.tile([128, 2, 1], f32, name=f"{R}bk_t")
    bv_t = wpool.tile([128, E], f32, name=f"{R}bv_t")
    ones_f = wpool.tile([1, 64], f32, name=f"{R}ones_f")
    ones_r = wpool.tile([1, 64], fr, name=f"{R}ones_r")
    onecol = wpool.tile([128, NS, G, 1], f32, name=f"{R}onecol")
    v_aug = wpool.tile([128, NS, G, HD + 1], fr, name=f"{R}v_aug")

    xT_t = dpool.tile([128, KC, T], bf16, name=f"{R}xT_t")
    hT_t = dpool.tile([128, KC, T], bf16, name=f"{R}hT_t")

    qT = [apool.tile([128, T], fr, name=f"{R}qT{p}") for p in range(2)]
    kT = [apool.tile([128, T], fr, name=f"{R}kT{p}") for p in range(2)]
    at2 = [apool.tile([128, T], fr, name=f"{R}at2_{p}") for p in range(2)]

    # ---- DMAs: t-sliced inputs so slice-0 compute starts early ----
    xT_r = xT_d[:].rearrange("(c p) t -> p c t", p=128)
    hT_r = hT_d[:].rearrange("(c p) t -> p c t", p=128)

    def sl(j):
        return slice(j * TB, (j + 1) * TB)

    wk_r = wk_d[:].rearrange("(c p) e -> p c e", p=128)
    wq_r = wq_d[:].rearrange("(c p) e -> p c e", p=128)
    nc.sync.dma_start(hT_t[:, :, sl(0)], hT_r[:, :, sl(0)])
    nc.sync.dma_start(wk_t[:, 0:4, :], wk_r[:, 0:4, :])
    nc.sync.dma_start(wq_t[:, 0:4, :], wq_r[:, 0:4, :])
    nc.sync.dma_start(xT_t[:, :, sl(0)], xT_r[:, :, sl(0)])
    nc.sync.dma_start(wk_t[:, 4:8, :], wk_r[:, 4:8, :])
    nc.sync.dma_start(wq_t[:, 4:8, :], wq_r[:, 4:8, :])
    nc.sync.dma_start(
        wv_t[:], wv_d[:].rearrange("(c p) e -> p c e", p=128))
    nc.sync.dma_start(bk_t[:], bk_d[:].rearrange("c p o -> p c o"))
    nc.sync.dma_start(bq_t[:], bq_d[:].rearrange("c p o -> p c o"))
    nc.sync.dma_start(bv_t[:], bv_d[:])
    nc.sync.dma_start(hT_t[:, :, sl(1)], hT_r[:, :, sl(1)])
    nc.sync.dma_start(xT_t[:, :, sl(1)], xT_r[:, :, sl(1)])
    nc.sync.dma_start(hT_t[:, :, sl(2)], hT_r[:, :, sl(2)])
    nc.sync.dma_start(xT_t[:, :, sl(2)], xT_r[:, :, sl(2)])
    nc.sync.dma_start(hT_t[:, :, sl(3)], hT_r[:, :, sl(3)])
    nc.sync.dma_start(xT_t[:, :, sl(3)], xT_r[:, :, sl(3)])
    nc.sync.dma_start(
        wo_t[:], wo_d[:].bitcast(fr).rearrange("(c p) d -> p c d", p=128))

    nc.vector.memset(ones_f[:], 1.0)
    nc.vector.tensor_copy(ones_r[:], ones_f[:])
    nc.vector.memset(onecol[:], 1.0)
    nc.vector.tensor_copy(v_aug[:, :, :, HD:HD + 1], onecol[:])

    # ---- work items (strictly sequential head-of-queue processing) ----
    done = set()

    def v_chunk(si):
        ps = qkps.tile([128, E], f32, name=f"{R}vp{si}", tag=f"qk{si % 2}")
        for k in range(KC):
            nc.tensor.matmul(
                ps[:], xT_t[:, k, si * 128:(si + 1) * 128], wv_t[:, k, :],
                start=(k == 0), stop=(k == KC - 1))
        nc.vector.tensor_tensor(
            v_aug[:, si, :, 0:HD],
            ps[:].rearrange("p (g e) -> p g e", g=G),
            bv_t[:].rearrange("p (g e) -> p g e", g=G),
            op=addop)
        done.add(("v", si))

    class Wave:
        """q/k projection wave for one t-slice; emits one k-step per call
        (one matmul per p in plist), then the bias-activations."""

        def __init__(self, which, j, plist, pool, tags):
            self.which, self.j, self.plist = which, j, plist
            self.w_t, self.b_t, self.dst = {
                "q": (wq_t, bq_t, qT), "k": (wk_t, bk_t, kT)}[which]
            self.ps = {p: pool.tile([128, TB], f32,
                                    name=f"{R}{which}{j}p{p}",
                                    tag=tags[i])
                       for i, p in enumerate(plist)}
            self.k = 0

        def step(self):
            if self.k < KC:
                for p in self.plist:
                    nc.tensor.matmul(
                        self.ps[p][:],
                        self.w_t[:, self.k, p * 128:(p + 1) * 128],
                        hT_t[:, self.k, sl(self.j)],
                        start=(self.k == 0), stop=(self.k == KC - 1))
                self.k += 1
                return 512 * len(self.plist)
            if self.k == KC:
                for p in self.plist:
                    # epilogue on DVE (not ACT): keeps the ACT queue
                    # exp-only so psum recycling never waits on exps
                    nc.vector.tensor_scalar_add(
                        self.dst[p][:, sl(self.j)], self.ps[p][:],
                        self.b_t[:, p, :])
                    done.add((self.which, self.j, p))
                self.k += 1
                return 64
            return -1

    class VFill:
        def __init__(self, si):
            self.si, self.emitted = si, False

        def step(self):
            if self.emitted:
                return -1
            v_chunk(self.si)
            self.emitted = True
            return 2048

    class Norm:
        """Deferred normalization of unit (tb,p): per call, one head's
        reciprocal-broadcast matmul + fused multiply into at2."""

        def __init__(self, u, tb, p, acc, rec):
            self.u, self.tb, self.p = u, tb, p
            self.acc, self.rec, self.h = acc, rec, 0
            self.defer = 2   # skip-turns before first emission so the
            #                  reciprocal has drained from the DVE queue

        def step(self):
            if self.h >= 2:
                return -1
            h, tb, p = self.h, self.tb, self.p
            bc = qkps.tile([64, TB], f32, name=f"{R}bc_{tb}_{p}_{h}",
                           tag=f"qk{h}")
            nc.tensor.matmul(bc[:], ones_r[:], self.rec[h][:],
                             start=True, stop=True)
            nc.vector.scalar_tensor_tensor(
                at2[p][h * 64:(h + 1) * 64, tb * TB:tb * TB + TB],
                self.acc[h][0:HD, :], 1.0, bc[:], op0=mult, op1=mult)
            self.h += 1
            if self.h == 2:
                done.add(("norm", self.u))
            return 512

    class OutProj:
        """out-projection for t-block tb; one (ts,dc) chunk per call:
        2 accum matmuls -> Pool copy to SBUF; DMA per completed ts."""

        def __init__(self, tb):
            self.tb, self.i = tb, 0
            self.osb = None

        def step(self):
            if self.i >= 8:
                return -1
            ts, dc = divmod(self.i, 2)
            t0 = self.tb * TB + ts * 128
            if dc == 0:
                self.osb = ospool.tile([128, D], f32,
                                       name=f"{R}osb{self.tb}_{ts}",
                                       tag="osb")
            ps = qkps.tile([128, 512], f32,
                           name=f"{R}op{self.tb}_{ts}_{dc}", tag=f"qk{dc}")
            for p in range(2):
                nc.tensor.matmul(
                    ps[:], at2[p][:, t0:t0 + 128],
                    wo_t[:, p, dc * 512:(dc + 1) * 512],
                    start=(p == 0), stop=(p == 1))
            nc.vector.tensor_copy(self.osb[:, dc * 512:(dc + 1) * 512],
                                  ps[:])
            if dc == 1:
                nc.sync.dma_start(out_d[t0:t0 + 128, :], self.osb[:])
            self.i += 1
            if self.i == 8:
                done.add(("out", self.tb))
            return 1024

    fillers = []
    tot = [0]

    def step_head(force=False):
        if not fillers:
            return False
        head = fillers[0]
        if getattr(head, "defer", 0) > 0:
            if not force:
                # postpone this item to a later pipeline step
                head.defer -= 1
                return False
            head.defer = 0
        r = head.step()
        if r < 0:
            fillers.pop(0)
        else:
            tot[0] += r
        return True

    def ensure(key):
        while key not in done:
            if not step_head(force=True):
                raise RuntimeError(f"work item {key} unavailable")

    def fill_to(target):
        while fillers and tot[0] < target:
            if not step_head():
                break

    # ---- ramp: slice-0 p0 projections so unit (0,0) starts ASAP ----
    rctx = ExitStack()
    rampps = rctx.enter_context(tc.tile_pool(name=f"{R}rampps", bufs=1,
                                             space="PSUM"))
    wk00 = Wave("k", 0, [0], rampps, ["t0"])
    wq00 = Wave("q", 0, [0], rampps, ["t1"])
    for _ in range(KC + 1):
        wk00.step()
    for _ in range(KC + 1):
        wq00.step()
    rctx.close()

    # ---- filler queue for the attention phase ----
    for si in range(0, 4):
        fillers.append(VFill(si))
    fillers.append(Wave("k", 0, [1], qkps, ["qk0"]))
    fillers.append(Wave("q", 0, [1], qkps, ["qk1"]))
    fillers.append(Wave("k", 1, [0, 1], qkps, ["qk0", "qk1"]))
    for si in range(4, 8):
        fillers.append(VFill(si))
    fillers.append(Wave("k", 2, [0, 1], qkps, ["qk0", "qk1"]))
    for si in range(8, 12):
        fillers.append(VFill(si))
    fillers.append(Wave("k", 3, [0, 1], qkps, ["qk0", "qk1"]))
    for si in range(12, 16):
        fillers.append(VFill(si))
    fillers.append(Wave("q", 1, [0, 1], qkps, ["qk0", "qk1"]))
    fillers.append(Wave("q", 2, [0, 1], qkps, ["qk0", "qk1"]))
    fillers.append(Wave("q", 3, [0, 1], qkps, ["qk0", "qk1"]))

    # ---- attention units, software-pipelined si loop ----
    cctx = ExitStack()
    scps = cctx.enter_context(tc.tile_pool(name=f"{R}scps", bufs=2,
                                           space="PSUM"))
    atps = cctx.enter_context(tc.tile_pool(name=f"{R}atps", bufs=1,
                                           space="PSUM"))

    def flush_block(acc, atp, first):
        """acc (SBUF) <- atp psum block (copy for b0, add for b1).
        The b0 copies split DVE/ACT so they run in parallel; the b1
        adds both go on DVE (GPSIMD cannot access PSUM)."""
        if first:
            nc.vector.tensor_copy(acc[0][:], atp[0][:])
            nc.scalar.copy(acc[1][:], atp[1][:])
        else:
            for h in range(2):
                nc.vector.tensor_tensor(acc[h][:], acc[h][:], atp[h][:],
                                        op=addop)

    # Global software pipeline over all units: at step g, emit scores for
    # s-chunk g and attn*V for s-chunk g-2 (lag 2 hides the PE->ACT->PE
    # round-trip latency of scores->exp->attn*V).
    units = [(tb, p) for tb in range(NT) for p in range(2)]
    NSC = len(units) * NS
    exs = [None, None, None]
    acc = atp = None
    for g in range(NSC + 2):
        base = tot[0]
        if g < NSC:
            u_sc, si = divmod(g, NS)
            tb, p = units[u_sc]
            ensure(("q", tb, p))
            ensure(("k", si // 4, p))
            scp = scps.tile([128, 2, TB], f32,
                            name=f"{R}sc_{tb}_{p}_{si}", tag="sc")
            for h in range(2):
                nc.tensor.matmul(
                    scp[:, h, :],
                    kT[p][h * 64:(h + 1) * 64,
                          si * 128:(si + 1) * 128],
                    qT[p][h * 64:(h + 1) * 64,
                          tb * TB:tb * TB + TB],
                    start=True, stop=True)
            ex = expool.tile([128, 2, TB], fr,
                             name=f"{R}ex_{tb}_{p}_{si}", tag="exp")
            nc.scalar.activation(ex[:], scp[:], Exp)
            exs[g % 3] = ex
        if g >= 2:
            u_at, sj = divmod(g - 2, NS)
            tb2, p2 = units[u_at]
            if sj == 0 and u_at >= 2:
                # acc/rec alias two generations back: the aliased unit's
                # Norm must be fully emitted before overwriting them
                ensure(("norm", u_at - 2))
            ensure(("v", sj))
        boost = 1024 if (g >= 2 and (g - 2) % 8 == 0) else 0
        fill_to(base + (2560 if g < 16 else 1024) + boost)
        if g >= 2:
            if sj % 8 == 0:
                if sj == 0:
                    acc = [accpool.tile([HD + 1, TB], f32,
                                        name=f"{R}acc_{tb2}_{p2}_{h}",
                                        tag=f"acc{h}")
                           for h in range(2)]
                else:
                    flush_block(acc, atp, first=True)
                atp = [atps.tile([HD + 1, TB], f32,
                                 name=f"{R}at_{tb2}_{p2}_{sj}_{h}",
                                 tag=f"at{h}")
                       for h in range(2)]
            for h in range(2):
                nc.tensor.matmul(
                    atp[h][:],
                    v_aug[:, sj, p2 * 2 + h, :],
                    exs[(g - 2) % 3][:, h, :],
                    start=(sj % 8 == 0), stop=(sj % 8 == 7),
                    skip_group_check=True)
            if sj == NS - 1:
                # unit end: flush b1 into acc, reciprocals, queue norm
                flush_block(acc, atp, first=False)
                rec = []
                for h in range(2):
                    rc = rpool.tile([1, TB], fr,
                                    name=f"{R}rc_{tb2}_{p2}_{h}",
                                    tag=f"rc{h}")
                    with nc.allow_low_precision(reason="f32r recip"):
                        nc.vector.reciprocal(rc[:],
                                             acc[h][HD:HD + 1, :])
                    rec.append(rc)
                fillers.append(Norm(u_at, tb2, p2, acc, rec))
                if p2 == 1:
                    fillers.append(OutProj(tb2))

    while step_head(force=True):
        pass

    cctx.close()
    octx.close()


def _get_program(reps=1):
    global _PROGRAM
    if _PROGRAM is None:
        _PROGRAM = {}
    if reps not in _PROGRAM:
        _PROGRAM[reps] = _build_program(reps)
    return _PROGRAM[reps]


def _shard_inputs(inputs):
    """Build the 8 per-core input maps from the full-problem inputs."""
    import ml_dtypes
    bf16 = ml_dtypes.bfloat16

    hs = np.asarray(inputs["hidden_states"], np.float32)
    pe = np.asarray(inputs["position_embeddings"], np.float32)
    Wq = np.asarray(inputs["Wq"], np.float32).reshape(D, H * HD)
    Wk = np.asarray(inputs["Wk"], np.float32).reshape(D, H * HD)
    Wv = np.asarray(inputs["Wv"], np.float32).reshape(D, H * HD)
    Wo = np.asarray(inputs["Wo"], np.float32)
    bq = np.asarray(inputs["bq"], np.float32).reshape(H * HD)
    bk = np.asarray(inputs["bk"], np.float32).reshape(H * HD)
    bv = np.asarray(inputs["bv"], np.float32).reshape(H * HD)

    hp = hs + pe
    xT = [np.ascontiguousarray(hs[b].T).astype(bf16) for b in range(B)]
    hT = [np.ascontiguousarray(hp[b].T).astype(bf16) for b in range(B)]

    in_maps = []
    for c in range(8):
        b, g = divmod(c, G)
        sel = slice(g * E, (g + 1) * E)
        in_maps.append({
            "xT": xT[b],
            "hT": hT[b],
            "wq": (np.ascontiguousarray(Wq[:, sel])
                   * np.float32(SCALE)).astype(bf16),
            "wk": np.ascontiguousarray(Wk[:, sel]).astype(bf16),
            "wv": np.ascontiguousarray(Wv[:, sel]).astype(bf16),
            "wo": np.ascontiguousarray(Wo[sel, :]),
            "bq": (bq[sel] * np.float32(SCALE)).reshape(2, 128, 1).copy(),
            "bk": bk[sel].reshape(2, 128, 1).copy(),
            "bvr": np.tile(bv[sel][None, :], (128, 1)),
        })
    return in_maps


def _gather_outputs(results, inputs):
    bo = np.asarray(inputs["bo"], np.float32)
    out = np.empty((B, S, D), np.float32)
    for b in range(B):
        acc = results[4 * b]["out"].astype(np.float32).copy()
        for g in range(1, G):
            acc += results[4 * b + g]["out"]
        out[b] = acc + bo[None, :]
    return out


def kernel(**inputs):
    from concourse.bass_utils import run_bass_kernel_spmd

    nc = _get_program()
    in_maps = _shard_inputs(inputs)
    res = run_bass_kernel_spmd(nc, in_maps, list(range(8)))
    return _gather_outputs(res.results, inputs)


# revision 49
# speedup vs baseline: 1.3181x; 1.0484x over previous
"""Trainium2 Bass kernel for DFine multi-head attention.

Problem: B=2, S=2048, D=1024, H=16 heads, HD=64.
Sharding over 8 cores: core c handles batch b=c//4 and head-group g=c%4
(4 heads). Each core computes its heads' attention and a partial
out-projection [2048, 1024]; the host sums the 4 partials per batch and
adds the output bias.

v3: one global software pipeline over all (t-block, head-pair) units:
at step g the PE emits scores for s-chunk g and attn*V for s-chunk
g-3; the 3-step lag (plus a 2-step deferral at each 8-chunk PSUM-block
boundary) hides the PE->ACT->PE round trip of scores->exp->attn*V and
the DVE block-flush latency. bf16 activations/weights from the host
(halves input DMA; h=x+pos precomputed on host), t-sliced input DMAs
so projections start at ~6us, projection / v / out-proj work
interleaved into the attention loop as PE filler via a keyed work
queue with a per-step row budget, attn*V accumulated in two 8-chunk
PSUM blocks flushed to an SBUF accumulator (DVE/ACT) so the next unit
never waits on the normalization chain, normalization fused into one
scalar_tensor_tensor, output partials stored as bf16 and summed on
the host.
"""

import sys
import numpy as np

if "/opt/trn_rl_repo" not in sys.path:
    sys.path.insert(0, "/opt/trn_rl_repo")

B, S, D, H, HD = 2, 2048, 1024, 16, 64
G = 4          # heads per core
E = G * HD     # 256 per-core head width
T = S          # tokens
KC = 8         # contraction chunks of 128 over D
TB = 512       # t-block (moving free dim)
NT = T // TB   # 4 t-blocks
NS = T // 128  # 16 s-chunks
SCALE = HD ** -0.5

# scheduling tunables (overridable for scans)
import os as _os
TUNE_BUDGET = int(_os.environ.get("TUNE_BUDGET", "1024"))
TUNE_EARLY = int(_os.environ.get("TUNE_EARLY", "1536"))
TUNE_BOOST = int(_os.environ.get("TUNE_BOOST", "1536"))
TUNE_DEFER = int(_os.environ.get("TUNE_DEFER", "8"))

_PROGRAM = None


def _build_program(reps=1):
    import concourse.bacc as bacc
    import concourse.tile as tile
    from concourse import mybir

    f32 = mybir.dt.float32
    bf16 = mybir.dt.bfloat16

    nc = bacc.Bacc("TRN2", target_bir_lowering=False, debug=False)

    xT_d = nc.declare_dram_parameter("xT", [D, T], bf16, isOutput=False)
    hT_d = nc.declare_dram_parameter("hT", [D, T], bf16, isOutput=False)
    wq_d = nc.declare_dram_parameter("wq", [D, E], bf16, isOutput=False)
    wk_d = nc.declare_dram_parameter("wk", [D, E], bf16, isOutput=False)
    wv_d = nc.declare_dram_parameter("wv", [D, E], bf16, isOutput=False)
    wo_d = nc.declare_dram_parameter("wo", [E, D], f32, isOutput=False)
    bq_d = nc.declare_dram_parameter("bq", [2, 128, 1], f32, isOutput=False)
    bk_d = nc.declare_dram_parameter("bk", [2, 128, 1], f32, isOutput=False)
    bv_d = nc.declare_dram_parameter("bvr", [128, E], f32, isOutput=False)
    out_d = nc.declare_dram_parameter("out", [T, D], bf16, isOutput=True)

    with tile.TileContext(nc) as tc:
        for rep in range(reps):
            _build_body(nc, tc, mybir, rep,
                        (xT_d, hT_d, wq_d, wk_d, wv_d, wo_d, bq_d, bk_d,
                         bv_d, out_d))

    nc.compile()
    return nc


def _build_body(nc, tc, mybir, rep, drams):
    from contextlib import ExitStack

    fr = mybir.dt.float32r
    f32 = mybir.dt.float32
    bf16 = mybir.dt.bfloat16
    Exp = mybir.ActivationFunctionType.Exp
    mult = mybir.AluOpType.mult
    addop = mybir.AluOpType.add
    (xT_d, hT_d, wq_d, wk_d, wv_d, wo_d, bq_d, bk_d, bv_d, out_d) = drams
    R = f"r{rep}_"

    octx = ExitStack()
    wpool = octx.enter_context(tc.tile_pool(name=f"{R}wpool", bufs=1))
    dpool = octx.enter_context(tc.tile_pool(name=f"{R}dpool", bufs=1))
    apool = octx.enter_context(tc.tile_pool(name=f"{R}apool", bufs=1))
    expool = octx.enter_context(tc.tile_pool(name=f"{R}expool", bufs=5))
    ospool = octx.enter_context(tc.tile_pool(name=f"{R}ospool", bufs=2))
    rpool = octx.enter_context(tc.tile_pool(name=f"{R}rpool", bufs=2))
    accpool = octx.enter_context(tc.tile_pool(name=f"{R}accpool", bufs=2))
    qkps = octx.enter_context(tc.tile_pool(name=f"{R}qkps", bufs=1,
                                           space="PSUM"))

    # ---- persistent tiles ----
    wq_t = wpool.tile([128, KC, E], bf16, name=f"{R}wq_t")
    wk_t = wpool.tile([128, KC, E], bf16, name=f"{R}wk_t")
    wv_t = wpool.tile([128, KC, E], bf16, name=f"{R}wv_t")
    wo_t = wpool.tile([128, 2, D], fr, name=f"{R}wo_t")
    bq_t = wpool.tile([128, 2, 1], f32, name=f"{R}bq_t")
    bk_t = wpool.tile([128, 2, 1], f32, name=f"{R}bk_t")
    bv_t = wpool.tile([128, E], f32, name=f"{R}bv_t")
    ones_f = wpool.tile([1, 64], f32, name=f"{R}ones_f")
    ones_r = wpool.tile([1, 64], fr, name=f"{R}ones_r")
    onecol = wpool.tile([128, NS, G, 1], f32, name=f"{R}onecol")
    v_aug = wpool.tile([128, NS, G, HD + 1], fr, name=f"{R}v_aug")

    xT_t = dpool.tile([128, KC, T], bf16, name=f"{R}xT_t")
    hT_t = dpool.tile([128, KC, T], bf16, name=f"{R}hT_t")

    qT = [apool.tile([128, T], fr, name=f"{R}qT{p}") for p in range(2)]
    kT = [apool.tile([128, T], fr, name=f"{R}kT{p}") for p in range(2)]
    at2 = [apool.tile([128, T], fr, name=f"{R}at2_{p}") for p in range(2)]

    # ---- DMAs: t-sliced inputs so slice-0 compute starts early ----
    xT_r = xT_d[:].rearrange("(c p) t -> p c t", p=128)
    hT_r = hT_d[:].rearrange("(c p) t -> p c t", p=128)

    def sl(j):
        return slice(j * TB, (j + 1) * TB)

    wk_r = wk_d[:].rearrange("(c p) e -> p c e", p=128)
    wq_r = wq_d[:].rearrange("(c p) e -> p c e", p=128)
    nc.sync.dma_start(hT_t[:, :, sl(0)], hT_r[:, :, sl(0)])
    nc.sync.dma_start(wk_t[:, 0:4, :], wk_r[:, 0:4, :])
    nc.sync.dma_start(wq_t[:, 0:4, :], wq_r[:, 0:4, :])
    nc.sync.dma_start(xT_t[:, :, sl(0)], xT_r[:, :, sl(0)])
    nc.sync.dma_start(wk_t[:, 4:8, :], wk_r[:, 4:8, :])
    nc.sync.dma_start(wq_t[:, 4:8, :], wq_r[:, 4:8, :])
    nc.sync.dma_start(
        wv_t[:], wv_d[:].rearrange("(c p) e -> p c e", p=128))
    nc.sync.dma_start(bk_t[:], bk_d[:].rearrange("c p o -> p c o"))
    nc.sync.dma_start(bq_t[:], bq_d[:].rearrange("c p o -> p c o"))
    nc.sync.dma_start(bv_t[:], bv_d[:])
    nc.sync.dma_start(hT_t[:, :, sl(1)], hT_r[:, :, sl(1)])
    nc.sync.dma_start(xT_t[:, :, sl(1)], xT_r[:, :, sl(1)])
    nc.sync.dma_start(hT_t[:, :, sl(2)], hT_r[:, :, sl(2)])
    nc.sync.dma_start(xT_t[:, :, sl(2)], xT_r[:, :, sl(2)])
    nc.sync.dma_start(hT_t[:, :, sl(3)], hT_r[:, :, sl(3)])
    nc.sync.dma_start(xT_t[:, :, sl(3)], xT_r[:, :, sl(3)])
    nc.sync.dma_start(
        wo_t[:], wo_d[:].bitcast(fr).rearrange("(c p) d -> p c d", p=128))

    nc.vector.memset(ones_f[:], 1.0)
    nc.vector.tensor_copy(ones_r[:], ones_f[:])
    nc.vector.memset(onecol[:], 1.0)
    nc.vector.tensor_copy(v_aug[:, :, :, HD:HD + 1], onecol[:])

    # ---- work items (strictly sequential head-of-queue processing) ----
    done = set()

    def v_chunk(si):
        ps = qkps.tile([128, E], f32, name=f"{R}vp{si}", tag=f"qk{si % 2}")
        for k in range(KC):
            nc.tensor.matmul(
                ps[:], xT_t[:, k, si * 128:(si + 1) * 128], wv_t[:, k, :],
                start=(k == 0), stop=(k == KC - 1))
        nc.vector.tensor_tensor(
            v_aug[:, si, :, 0:HD],
            ps[:].rearrange("p (g e) -> p g e", g=G),
            bv_t[:].rearrange("p (g e) -> p g e", g=G),
            op=addop)
        done.add(("v", si))

    class Wave:
        """q/k projection wave for one t-slice; emits one k-step per call
        (one matmul per p in plist), then the bias-activations."""

        def __init__(self, which, j, plist, pool, tags):
            self.which, self.j, self.plist = which, j, plist
            self.w_t, self.b_t, self.dst = {
                "q": (wq_t, bq_t, qT), "k": (wk_t, bk_t, kT)}[which]
            self.pool, self.tags = pool, tags
            self.ps = None
            self.k = 0

        def step(self):
            if self.ps is None:
                # allocate lazily at first emission: pool generation
                # order must match processing order, not build order
                self.ps = {p: self.pool.tile([128, TB], f32,
                                             name=f"{R}{self.which}"
                                                  f"{self.j}p{p}",
                                             tag=self.tags[i])
                           for i, p in enumerate(self.plist)}
            if self.k < KC:
                for p in self.plist:
                    nc.tensor.matmul(
                        self.ps[p][:],
                        self.w_t[:, self.k, p * 128:(p + 1) * 128],
                        hT_t[:, self.k, sl(self.j)],
                        start=(self.k == 0), stop=(self.k == KC - 1))
                self.k += 1
                return 512 * len(self.plist)
            if self.k == KC:
                for p in self.plist:
                    # epilogue on DVE (not ACT): keeps the ACT queue
                    # exp-only so psum recycling never waits on exps
                    nc.vector.tensor_scalar_add(
                        self.dst[p][:, sl(self.j)], self.ps[p][:],
                        self.b_t[:, p, :])
                    done.add((self.which, self.j, p))
                self.k += 1
                return 64
            return -1

    class VFill:
        def __init__(self, si):
            self.si, self.emitted = si, False

        def step(self):
            if self.emitted:
                return -1
            v_chunk(self.si)
            self.emitted = True
            return 2048

    class Norm:
        """Deferred normalization of unit (tb,p): per call, one head's
        reciprocal-broadcast matmul + fused multiply into at2."""

        def __init__(self, u, tb, p, acc, rec):
            self.u, self.tb, self.p = u, tb, p
            self.acc, self.rec, self.h = acc, rec, 0
            self.defer = TUNE_DEFER   # skip-turns before first emission so the
            #                  reciprocal has drained from the DVE queue

        def step(self):
            if self.h >= 2:
                return -1
            h, tb, p = self.h, self.tb, self.p
            bc = qkps.tile([64, TB], f32, name=f"{R}bc_{tb}_{p}_{h}",
                           tag=f"qk{h}")
            nc.tensor.matmul(bc[:], ones_r[:], self.rec[h][:],
                             start=True, stop=True)
            nc.vector.scalar_tensor_tensor(
                at2[p][h * 64:(h + 1) * 64, tb * TB:tb * TB + TB],
                self.acc[h][0:HD, :], 1.0, bc[:], op0=mult, op1=mult)
            self.h += 1
            if self.h == 2:
                done.add(("norm", self.u))
            return 512

    class OutProj:
        """out-projection for t-block tb; one (ts,dc) chunk per call:
        2 accum matmuls -> copy to SBUF -> DMA. `split` alternates the
        copies between DVE and ACT (used for the final t-block where the
        serialized DVE copies would otherwise be the kernel tail)."""

        def __init__(self, tb, split=False):
            self.tb, self.i, self.split = tb, 0, split
            self.osb = None

        def step(self):
            if self.i >= 8:
                return -1
            ts, dc = divmod(self.i, 2)
            t0 = self.tb * TB + ts * 128
            if dc == 0:
                self.osb = ospool.tile([128, D], bf16,
                                       name=f"{R}osb{self.tb}_{ts}",
                                       tag="osb")
            ps = qkps.tile([128, 512], f32,
                           name=f"{R}op{self.tb}_{ts}_{dc}", tag=f"qk{dc}")
            for p in range(2):
                nc.tensor.matmul(
                    ps[:], at2[p][:, t0:t0 + 128],
                    wo_t[:, p, dc * 512:(dc + 1) * 512],
                    start=(p == 0), stop=(p == 1))
            dst = self.osb[:, dc * 512:(dc + 1) * 512]
            if self.split and dc == 1:
                nc.scalar.copy(dst, ps[:])
            else:
                nc.vector.tensor_copy(dst, ps[:])
            nc.sync.dma_start(
                out_d[t0:t0 + 128, dc * 512:(dc + 1) * 512], dst)
            self.i += 1
            if self.i == 8:
                done.add(("out", self.tb))
            return 1024

    fillers = []
    tot = [0]

    no_out = [False]   # blackout: keep OutProj DVE copies away from
    #                    unit boundaries where flushes gate attn*V

    def step_head(force=False):
        if not fillers:
            return False
        head = fillers[0]
        if getattr(head, "defer", 0) > 0:
            if not force:
                # postpone this item to a later pipeline step
                head.defer -= 1
                return False
            head.defer = 0
        if isinstance(head, OutProj) and no_out[0] and not force:
            return False
        r = head.step()
        if r < 0:
            fillers.pop(0)
        else:
            tot[0] += r
        return True

    def ensure(key):
        while key not in done:
            if not step_head(force=True):
                raise RuntimeError(f"work item {key} unavailable")

    def fill_to(target):
        while fillers and tot[0] < target:
            if not step_head():
                break

    # ---- ramp: slice-0 p0 projections so unit (0,0) starts ASAP ----
    rctx = ExitStack()
    rampps = rctx.enter_context(tc.tile_pool(name=f"{R}rampps", bufs=1,
                                             space="PSUM"))
    wk00 = Wave("k", 0, [0], rampps, ["t0"])
    wq00 = Wave("q", 0, [0], rampps, ["t1"])
    for _ in range(KC + 1):
        wk00.step()
    for _ in range(KC + 1):
        wq00.step()
    rctx.close()

    # ---- filler queue for the attention phase ----
    for si in range(0, 4):
        fillers.append(VFill(si))
    fillers.append(Wave("k", 0, [1], qkps, ["qk0"]))
    fillers.append(Wave("q", 0, [1], qkps, ["qk1"]))
    fillers.append(Wave("k", 1, [0, 1], qkps, ["qk0", "qk1"]))
    for si in range(4, 8):
        fillers.append(VFill(si))
    fillers.append(Wave("k", 2, [0, 1], qkps, ["qk0", "qk1"]))
    for si in range(8, 12):
        fillers.append(VFill(si))
    fillers.append(Wave("k", 3, [0, 1], qkps, ["qk0", "qk1"]))
    for si in range(12, 16):
        fillers.append(VFill(si))
    fillers.append(Wave("q", 1, [0, 1], qkps, ["qk0", "qk1"]))
    fillers.append(Wave("q", 2, [0, 1], qkps, ["qk0", "qk1"]))
    fillers.append(Wave("q", 3, [0, 1], qkps, ["qk0", "qk1"]))

    # ---- attention units, software-pipelined si loop ----
    cctx = ExitStack()
    scps = cctx.enter_context(tc.tile_pool(name=f"{R}scps", bufs=2,
                                           space="PSUM"))
    atps = cctx.enter_context(tc.tile_pool(name=f"{R}atps", bufs=1,
                                           space="PSUM"))

    def flush_block(acc, atp, first):
        """acc (SBUF) <- atp psum block (copy for b0, add for b1).
        The b0 copies split DVE/ACT so they run in parallel; the b1
        adds both go on DVE (GPSIMD cannot access PSUM)."""
        if first:
            nc.vector.tensor_copy(acc[0][:], atp[0][:])
            nc.scalar.copy(acc[1][:], atp[1][:])
        else:
            for h in range(2):
                nc.vector.tensor_tensor(acc[h][:], acc[h][:], atp[h][:],
                                        op=addop)

    # Global software pipeline over all units: at step g, emit scores for
    # s-chunk g and attn*V for s-chunk g-2 (lag 2 hides the PE->ACT->PE
    # round-trip latency of scores->exp->attn*V).
    units = [(tb, p) for tb in range(NT) for p in range(2)]
    NSC = len(units) * NS
    LAG = 3
    exs = [None] * 6
    acc = atp = None
    at_pending = []     # attn*V deferred two steps at each block start
    for g in range(NSC + LAG):
        base = tot[0]
        if g < NSC:
            u_sc, si = divmod(g, NS)
            tb, p = units[u_sc]
            ensure(("q", tb, p))
            ensure(("k", si // 4, p))
            scp = scps.tile([128, 2, TB], f32,
                            name=f"{R}sc_{tb}_{p}_{si}", tag="sc")
            for h in range(2):
                nc.tensor.matmul(
                    scp[:, h, :],
                    kT[p][h * 64:(h + 1) * 64,
                          si * 128:(si + 1) * 128],
                    qT[p][h * 64:(h + 1) * 64,
                          tb * TB:tb * TB + TB],
                    start=True, stop=True)
            ex = expool.tile([128, 2, TB], fr,
                             name=f"{R}ex_{tb}_{p}_{si}", tag="exp")
            nc.scalar.activation(ex[:], scp[:], Exp)
            exs[g % 6] = ex
        if g >= LAG:
            u_at, sj = divmod(g - LAG, NS)
            tb2, p2 = units[u_at]
            if sj == 0 and u_at >= 2:
                # acc/rec alias two generations back: the aliased unit's
                # Norm must be fully emitted before overwriting them
                ensure(("norm", u_at - 2))
            ensure(("v", sj))
        no_out[0] = g >= LAG and ((g - LAG) % NS) in (13, 14, 15, 0, 1)
        boost = TUNE_BOOST if (g >= LAG and (g - LAG) % 8 == 0) else 0
        fill_to(base + (TUNE_EARLY if g < 16 else TUNE_BUDGET) + boost)
        if g >= LAG:
            if sj % 8 == 0:
                if sj == 0:
                    acc = [accpool.tile([HD + 1, TB], f32,
                                        name=f"{R}acc_{tb2}_{p2}_{h}",
                                        tag=f"acc{h}")
                           for h in range(2)]
                else:
                    flush_block(acc, atp, first=True)
                atp = [atps.tile([HD + 1, TB], f32,
                                 name=f"{R}at_{tb2}_{p2}_{sj}_{h}",
                                 tag=f"at{h}")
                       for h in range(2)]

            def emit_at(sj_, exi):
                for h in range(2):
                    nc.tensor.matmul(
                        atp[h][:],
                        v_aug[:, sj_, p2 * 2 + h, :],
                        exs[exi][:, h, :],
                        start=(sj_ % 8 == 0), stop=(sj_ % 8 == 7),
                        skip_group_check=True)

            if sj % 8 in (0, 1):
                # defer the block's first attn*V pairs two steps so the
                # PSUM-block flush fully drains before the bank is reused
                at_pending.append((sj, (g - LAG) % 6))
            else:
                for item in at_pending:
                    emit_at(*item)
                at_pending.clear()
                emit_at(sj, (g - LAG) % 6)
            if sj == NS - 1:
                # unit end: flush b1 into acc, reciprocals, queue norm
                flush_block(acc, atp, first=False)
                rec = []
                for h in range(2):
                    rc = rpool.tile([1, TB], fr,
                                    name=f"{R}rc_{tb2}_{p2}_{h}",
                                    tag=f"rc{h}")
                    with nc.allow_low_precision(reason="f32r recip"):
                        nc.vector.reciprocal(rc[:],
                                             acc[h][HD:HD + 1, :])
                    rec.append(rc)
                fillers.append(Norm(u_at, tb2, p2, acc, rec))
                if p2 == 1:
                    fillers.append(OutProj(tb2, split=(u_at == 7)))

    while step_head(force=True):
        pass

    cctx.close()
    octx.close()


def _get_program(reps=1):
    global _PROGRAM
    if _PROGRAM is None:
        _PROGRAM = {}
    if reps not in _PROGRAM:
        _PROGRAM[reps] = _build_program(reps)
    return _PROGRAM[reps]


def _shard_inputs(inputs):
    """Build the 8 per-core input maps from the full-problem inputs."""
    import ml_dtypes
    bf16 = ml_dtypes.bfloat16

    hs = np.asarray(inputs["hidden_states"], np.float32)
    pe = np.asarray(inputs["position_embeddings"], np.float32)
    Wq = np.asarray(inputs["Wq"], np.float32).reshape(D, H * HD)
    Wk = np.asarray(inputs["Wk"], np.float32).reshape(D, H * HD)
    Wv = np.asarray(inputs["Wv"], np.float32).reshape(D, H * HD)
    Wo = np.asarray(inputs["Wo"], np.float32)
    bq = np.asarray(inputs["bq"], np.float32).reshape(H * HD)
    bk = np.asarray(inputs["bk"], np.float32).reshape(H * HD)
    bv = np.asarray(inputs["bv"], np.float32).reshape(H * HD)

    hp = hs + pe
    xT = [np.ascontiguousarray(hs[b].T).astype(bf16) for b in range(B)]
    hT = [np.ascontiguousarray(hp[b].T).astype(bf16) for b in range(B)]

    in_maps = []
    for c in range(8):
        b, g = divmod(c, G)
        sel = slice(g * E, (g + 1) * E)
        in_maps.append({
            "xT": xT[b],
            "hT": hT[b],
            "wq": (np.ascontiguousarray(Wq[:, sel])
                   * np.float32(SCALE)).astype(bf16),
            "wk": np.ascontiguousarray(Wk[:, sel]).astype(bf16),
            "wv": np.ascontiguousarray(Wv[:, sel]).astype(bf16),
            "wo": np.ascontiguousarray(Wo[sel, :]),
            "bq": (bq[sel] * np.float32(SCALE)).reshape(2, 128, 1).copy(),
            "bk": bk[sel].reshape(2, 128, 1).copy(),
            "bvr": np.tile(bv[sel][None, :], (128, 1)),
        })
    return in_maps


def _gather_outputs(results, inputs):
    bo = np.asarray(inputs["bo"], np.float32)
    out = np.empty((B, S, D), np.float32)
    for b in range(B):
        acc = results[4 * b]["out"].astype(np.float32).copy()
        for g in range(1, G):
            acc += results[4 * b + g]["out"]
        out[b] = acc + bo[None, :]
    return out


def kernel(**inputs):
    from concourse.bass_utils import run_bass_kernel_spmd

    nc = _get_program()
    in_maps = _shard_inputs(inputs)
    res = run_bass_kernel_spmd(nc, in_maps, list(range(8)))
    return _gather_outputs(res.results, inputs)
